# revision 1
# baseline (speedup 1.0000x reference)
"""Trainium2 Bass kernel for batched tanh-RNN (B=5000, T=8, V=5264, H=200).

  xh   = X @ W_ih.T + b_ih + b_hh          # [B, T, H]  (bulk of FLOPs)
  h_t  = tanh(xh[:, t] + h_{t-1} @ W_hh.T) # 8 steps
  out  = h_T @ W_fc.T + b_fc               # [B, V]

Strategy: data-parallel over batch across 8 NeuronCores (625 rows each),
weights replicated.  On each core everything is computed in "transposed"
layout (hidden dim on partitions, batch on the free dim) so the recurrence
needs no on-chip transposes:

  phase 1: xh.T[h, t*625+b] accumulated in PSUM over 42 v-tiles of 128;
           stationary = W_ih.T tiles, moving = X.T slabs streamed from HBM.
           X is re-laid-out on the host to [128, 42, 5000] (v-major) so the
           DMA is 2KB-contiguous per partition.
  phase 2: h.T = tanh(W_hh.T.T @ h.T + xh_t.T); the xh_t term is added into
           the same PSUM accumulation group via an identity-stationary
           matmul, then one ACT Tanh PSUM->SBUF per tile.
  phase 3: out[b, v] = h.T-as-stationary @ W_fc.T tiles (+ b_fc via a
           ones-stationary K=1 matmul), natural-layout DMA store.

All matmul operands are float32r (FP22 single-pass, 1 cycle/row for moving
free dim >= 256) — ~1e-4 relative precision, full PE speed.
"""

import numpy as np

import concourse.bass as bass
import concourse.mybir as mybir
from concourse import bacc
from concourse.bass_utils import run_bass_kernel_spmd
from concourse.tile import TileContext

NCORE = 8
B, T, V, H = 5000, 8, 5264, 200
Bc = B // NCORE            # 625 batch rows per core
Bp = 640                   # padded to keep all fp32r APs 8-byte aligned
BT = Bp * T                # 5120 (t-major columns: col = t*Bp + b)
VP = 5376                  # V padded to 42*128
KT = VP // 128             # 42 contraction tiles
SUB = 14                   # v-tiles per streamed X slab
NSUB = KT // SUB           # 3 slabs per bt-chunk
CH = 512                   # phase-1 bt-chunk width (PSUM bank = 512 fp32)
NCH = BT // CH             # 10 chunks
HA, HB = 128, H - 128      # hidden split across partition tiles (128 + 72)

F32 = mybir.dt.float32
F32R = mybir.dt.float32r
BF16 = mybir.dt.bfloat16
AF = mybir.ActivationFunctionType

# Stream X (and W_ih) in bf16: halves the dominant HBM traffic; phase-1
# products are bf16*bf16 -> fp32 PSUM. Everything downstream stays fp32.
X_BF16 = True

# recurrence b-chunks (even, >=256 so float32r runs 1 cycle/row)
REC_CHUNKS = [(0, 320), (320, 320)]
# FC output v-chunks (even offsets/widths, all >=256, <=512)
FC_CHUNKS = [(i * 480, 480) for i in range(10)] + [(4800, 464)]
# FC batch tiles over padded 640 (stationary free dim = 128; the last tile
# computes 15 pad rows that are simply not stored)
FC_BTILES = [(0, 128, 128), (128, 128, 128), (256, 128, 128),
             (384, 128, 128), (512, 128, 113)]

_CACHE = {}
LAST_RESULT = None  # BassKernelResults of the most recent run (for test.py)


def _build(reps=1, bench_internal=False, phases=3, sub=SUB, xbufs=3, fc_mode='full'):
    # Bacc (not raw Bass): its finalize() runs move_matmul_waits_to_ldweights
    # + generate_event_semaphores, required on TRN2 (max 1 sync wait/inst).
    # reps>1 re-emits the whole body (idempotent) for slope-based HW timing.
    # bench_internal keeps the big inputs as Internal DRAM (no upload per
    # call; contents garbage — timing is data-independent).
    nc = bacc.Bacc()

    xdt = BF16 if X_BF16 else F32R
    if bench_internal:
        XT = nc.dram_tensor("XT", [128, KT, BT], xdt)
        H0T = nc.dram_tensor("H0T", [H, Bp], F32R)
        WIH = nc.dram_tensor("WIH", [128, KT, H], xdt)
        WHH = nc.dram_tensor("WHH", [H, H], F32R)
        BIASH = nc.dram_tensor("BIASH", [H, 1], F32)
        WFC = nc.dram_tensor("WFC", [H, V], BF16)
        BFC = nc.dram_tensor("BFC", [1, V], BF16)
    else:
        XT = nc.declare_dram_parameter("XT", [128, KT, BT], xdt, isOutput=False)
        H0T = nc.declare_dram_parameter("H0T", [H, Bp], F32R, isOutput=False)
        WIH = nc.declare_dram_parameter("WIH", [128, KT, H], xdt, isOutput=False)
        WHH = nc.declare_dram_parameter("WHH", [H, H], F32R, isOutput=False)
        BIASH = nc.declare_dram_parameter("BIASH", [H, 1], F32, isOutput=False)
        WFC = nc.declare_dram_parameter("WFC", [H, V], BF16, isOutput=False)
        BFC = nc.declare_dram_parameter("BFC", [1, V], BF16, isOutput=False)
    IDEN = nc.declare_dram_parameter("IDEN", [128, 128], F32R, isOutput=False)
    ONES = nc.declare_dram_parameter("ONES", [1, 128], BF16, isOutput=False)
    YOUT = nc.declare_dram_parameter("YOUT", [Bc, V], F32, isOutput=True)

    with TileContext(nc) as tc:
      for _rep in range(reps):
        with tc.tile_pool(name="const", bufs=1) as cpool, \
             tc.tile_pool(name="hpool", bufs=2) as hpool:
            whh_a = cpool.tile([HA, H], F32R, tag="whh_a")
            whh_b = cpool.tile([HB, H], F32R, tag="whh_b")
            biash_a = cpool.tile([HA, 1], F32, tag="biash_a")
            biash_b = cpool.tile([HB, 1], F32, tag="biash_b")
            iden = cpool.tile([128, 128], F32R, tag="iden")
            ones = cpool.tile([1, 128], BF16, tag="ones")
            xh_a = cpool.tile([HA, BT], F32R, tag="xh_a")
            xh_b = cpool.tile([HB, BT], F32R, tag="xh_b")

            nc.gpsimd.dma_start(out=whh_a, in_=WHH[0:HA, :])
            nc.gpsimd.dma_start(out=whh_b, in_=WHH[HA:H, :])
            nc.gpsimd.dma_start(out=biash_a, in_=BIASH[0:HA, :])
            nc.gpsimd.dma_start(out=biash_b, in_=BIASH[HA:H, :])
            nc.gpsimd.dma_start(out=iden, in_=IDEN[:, :])
            nc.gpsimd.dma_start(out=ones, in_=ONES[:, :])

            cur_a = hpool.tile([HA, Bp], F32R, tag="ha")
            cur_b = hpool.tile([HB, Bp], F32R, tag="hb")
            nc.gpsimd.dma_start(out=cur_a, in_=H0T[0:HA, :])
            nc.gpsimd.dma_start(out=cur_b, in_=H0T[HA:H, :])

            # FC weights (bf16) load up-front on the gpsimd ring so they
            # are resident long before phase 3 starts.
            fpool = tc.alloc_tile_pool(name="fc", bufs=1)
            wfc_a = fpool.tile([HA, V], BF16, tag="wfc_a")
            wfc_b = fpool.tile([HB, V], BF16, tag="wfc_b")
            bfc_sb = fpool.tile([1, V], BF16, tag="bfc")
            nc.gpsimd.dma_start(out=wfc_a, in_=WFC[0:HA, :])
            nc.gpsimd.dma_start(out=wfc_b, in_=WFC[HA:H, :])
            nc.gpsimd.dma_start(out=bfc_sb, in_=BFC[:, :])

            # ---- phase 1: xh.T = (W_ih.T).T @ X.T + bias, PSUM-accumulated
            with tc.tile_pool(name="wih", bufs=1) as wpool, \
                 tc.tile_pool(name="xslab", bufs=xbufs) as xpool, \
                 tc.tile_pool(name="ps1", bufs=2, space="PSUM") as ps1:
                wih_sb = wpool.tile([128, KT, H], xdt, tag="wih")
                nc.gpsimd.dma_start(out=wih_sb, in_=WIH[:, :, :])

                for c in range(NCH if phases != 4 else 0):
                    pa = ps1.tile([HA, CH], F32, tag="pa")
                    pb = ps1.tile([HB, CH], F32, tag="pb")
                    for s in range(KT // sub):
                        xs = xpool.tile([128, sub, CH], xdt, tag="xs")
                        nc.sync.dma_start(
                            out=xs,
                            in_=XT[:, s * sub:(s + 1) * sub, c * CH:(c + 1) * CH],
                        )
                        for j in range(sub):
                            k = s * sub + j
                            st = (k == 0)
                            sp = (k == KT - 1)
                            nc.tensor.matmul(
                                pa, wih_sb[:, k, 0:HA], xs[:, j, :],
                                start=st, stop=sp,
                            )
                            nc.tensor.matmul(
                                pb, wih_sb[:, k, HA:H], xs[:, j, :],
                                start=st, stop=sp,
                            )
                    nc.scalar.activation(
                        xh_a[:, c * CH:(c + 1) * CH], pa, AF.Identity,
                        bias=biash_a,
                    )
                    nc.scalar.activation(
                        xh_b[:, c * CH:(c + 1) * CH], pb, AF.Identity,
                        bias=biash_b,
                    )

            # ---- phase 2: 8 recurrence steps, h kept as [h, b] tiles
            with tc.tile_pool(name="ps2", bufs=2, space="PSUM") as ps2:
              if phases == 2 or phases == 3:
                  for t in range(T):
                      new_a = hpool.tile([HA, Bp], F32R, tag="ha")
                      new_b = hpool.tile([HB, Bp], F32R, tag="hb")
                      for (c0, cn) in REC_CHUNKS:
                          p0 = ps2.tile([HA, 320], F32, tag="p0")
                          p1 = ps2.tile([HB, 320], F32, tag="p1")
                          col = t * Bp + c0
                          # h_new[0:128]
                          nc.tensor.matmul(
                              p0[:, 0:cn], whh_a[:, 0:HA], cur_a[:, c0:c0 + cn],
                              start=True, stop=False)
                          nc.tensor.matmul(
                              p0[:, 0:cn], whh_b[:, 0:HA], cur_b[:, c0:c0 + cn],
                              start=False, stop=False)
                          nc.tensor.matmul(
                              p0[:, 0:cn], iden, xh_a[:, col:col + cn],
                              start=False, stop=True)
                          # h_new[128:200]
                          nc.tensor.matmul(
                              p1[:, 0:cn], whh_a[:, HA:H], cur_a[:, c0:c0 + cn],
                              start=True, stop=False)
                          nc.tensor.matmul(
                              p1[:, 0:cn], whh_b[:, HA:H], cur_b[:, c0:c0 + cn],
                              start=False, stop=False)
                          nc.tensor.matmul(
                              p1[:, 0:cn], iden[0:HB, 0:HB], xh_b[:, col:col + cn],
                              start=False, stop=True)
                          nc.scalar.activation(
                              new_a[:, c0:c0 + cn], p0[:, 0:cn], AF.Tanh)
                          nc.scalar.activation(
                              new_b[:, c0:c0 + cn], p1[:, 0:cn], AF.Tanh)
                      cur_a, cur_b = new_a, new_b

            # ---- phase 3: out = h_last @ W_fc.T + b_fc, natural layout
            if phases < 3:
                # still touch YOUT so outputs exist (gpsimd can cast f32r->f32)
                nc.gpsimd.dma_start(out=YOUT[0:HA, 0:Bp], in_=cur_a)
                fpool.release()
                continue
            with tc.tile_pool(name="outp", bufs=2) as opool, \
                 tc.tile_pool(name="ps3", bufs=4, space="PSUM") as ps3:
                # cast h_last to bf16 so FC stationaries use the fast
                # (FWL) weight-load path instead of ~1.1us fp32 self-loads
                h8_a = opool.tile([HA, Bp], BF16, tag="h8a", bufs=1)
                h8_b = opool.tile([HB, Bp], BF16, tag="h8b", bufs=1)
                nc.vector.tensor_copy(h8_a, cur_a)
                nc.vector.tensor_copy(h8_b, cur_b)

                for bi, (b0, bn, bs) in enumerate(FC_BTILES):
                    yt = opool.tile([128, V], F32, tag="yt")
                    for (v0, vn) in FC_CHUNKS:
                        pf = ps3.tile([128, 480], F32, tag="pf")
                        nc.tensor.matmul(
                            pf[0:bn, 0:vn], h8_a[:, b0:b0 + bn],
                            wfc_a[:, v0:v0 + vn], start=True,
                            stop=(fc_mode == 'mm1'))
                        if fc_mode != 'mm1':
                            nc.tensor.matmul(
                                pf[0:bn, 0:vn], h8_b[:, b0:b0 + bn],
                                wfc_b[:, v0:v0 + vn], start=False, stop=False)
                            nc.tensor.matmul(
                                pf[0:bn, 0:vn], ones[:, 0:bn],
                                bfc_sb[:, v0:v0 + vn], start=False, stop=True)
                        if fc_mode in ('nostore', 'full'):
                            nc.vector.tensor_copy(
                                yt[0:bn, v0:v0 + vn], pf[0:bn, 0:vn])
                        else:
                            nc.vector.tensor_copy(
                                yt[0:bn, 0:8], pf[0:bn, 0:8])
                    if fc_mode == 'full':
                        # one big store per b-tile, alternating HWDGE rings
                        eng = nc.sync if bi % 2 == 0 else nc.scalar
                        eng.dma_start(out=YOUT[b0:b0 + bs, :], in_=yt[0:bs, :])
                    else:
                        nc.scalar.dma_start(out=YOUT[b0:b0 + bs, 0:8],
                                            in_=yt[0:bs, 0:8])
            fpool.release()

    nc.finalize()
    return nc


def _prep_host(X, h0, W_ih, W_hh, b_ih, b_hh, W_fc, b_fc):
    f = np.float32
    import ml_dtypes
    xf = ml_dtypes.bfloat16 if X_BF16 else f
    X = np.asarray(X, f)
    # X.T slabs: XTr[core, p, k, t*Bp+b] = X[core*Bc+b, t, k*128+p]
    # (v zero-padded to VP, b zero-padded to Bp)
    srcp = np.zeros((NCORE, VP, T, Bp), xf)
    srcp[:, :V, :, :Bc] = X.reshape(NCORE, Bc, T, V).transpose(0, 3, 2, 1)
    srcp = srcp.reshape(NCORE, VP, BT)
    XTr = np.ascontiguousarray(srcp.reshape(NCORE, KT, 128, BT).transpose(0, 2, 1, 3))
    del srcp

    wih_t = np.zeros((VP, H), xf)
    wih_t[:V] = np.asarray(W_ih, f).T                      # [v, h]
    WIHr = np.ascontiguousarray(wih_t.reshape(KT, 128, H).transpose(1, 0, 2))

    WHHt = np.ascontiguousarray(np.asarray(W_hh, f).T)     # [h_prev, h_new]
    BIASHv = (np.asarray(b_ih, f) + np.asarray(b_hh, f)).reshape(H, 1).copy()
    WFCt = np.ascontiguousarray(np.asarray(W_fc, ml_dtypes.bfloat16).T)  # [h, v]
    BFCv = np.asarray(b_fc, ml_dtypes.bfloat16).reshape(1, V).copy()
    H0T = np.zeros((NCORE, H, Bp), f)
    H0T[:, :, :Bc] = np.asarray(h0, f).reshape(NCORE, Bc, H).transpose(0, 2, 1)
    IDENv = np.eye(128, dtype=f)
    ONESv = np.ones((1, 128), ml_dtypes.bfloat16)

    in_maps = []
    for i in range(NCORE):
        in_maps.append({
            "XT": XTr[i], "H0T": H0T[i], "WIH": WIHr, "WHH": WHHt,
            "BIASH": BIASHv, "WFC": WFCt, "BFC": BFCv, "IDEN": IDENv,
            "ONES": ONESv,
        })
    return in_maps


def kernel(X, h0, W_ih, W_hh, b_ih, b_hh, W_fc, b_fc):
    global LAST_RESULT
    in_maps = _prep_host(X, h0, W_ih, W_hh, b_ih, b_hh, W_fc, b_fc)
    if "nc" not in _CACHE:
        _CACHE["nc"] = _build()
    res = run_bass_kernel_spmd(_CACHE["nc"], in_maps, list(range(NCORE)))
    LAST_RESULT = res
    return np.concatenate([res.results[i]["YOUT"] for i in range(NCORE)], axis=0)

